# revision 51
# baseline (speedup 1.0000x reference)
"""Trainium2 Bass kernel for nn_InvariantHeadviaTP.

Reference computation (after dead-code elimination -- y1/y2/gates are never
used by the output):
    x0   = node_vec[:, :128]                  # [N, 128]
    a    = node_embedding                     # [N, 16]
    s0   = einsum('ni,na,iak->nk', x0, a, W1_l0[:, :, :128]) / sqrt(2048) + b1[:128]
    scal = silu(s0)                           # [N, 128]
    mid  = einsum('ni,na,iak->nk', scal, a, W2) / sqrt(2048) + b2   # [N, 16]
    h    = silu(mid @ W3 / 4 + b3)            # [N, 16]
    out  = h @ W4 / 4 + b4                    # [N, 1]

Strategy: data-parallel over 8 cores (2048 nodes each), transposed layout
(features on partitions, nodes on the free dim), FREE=512 node tiles.

The bilinear contraction over c=(a,i) [16*128=2048] is chunked as 16 blocks
of (4 a's x 32 i's).  Each chunk's elementwise factor tile is then either
  ar4_q[p,n]  = aT[4q + p//32, n]   (4 distinct tiles, a replicated 32x)
  x0r_h[p,n]  = x0T[32h + p%32, n]  (4 distinct tiles, x0 rows tiled 4x)
so only 8 broadcast-inflated [128,nsh] tensors ship from HBM (4 MB/core)
instead of the 16 a-only ones (8 MB/core).  U chunks are built with 4
conglomerate DVE multiplies per tile (stride-0 / strided 3D APs), then 16
PSUM-accumulated matmuls at full 128x128 PE utilization give s0.

The mid path is reassociated: M = (W2*SCALE) @ (W3/sqrt(A)) folded on host,
Q'[(a,j),n] = M~^T scal (2 matmuls), R' = Q' * ar16 (DVE), then a one-hot
selector matmul sums over a straight into h_pre [16,n] (W3 already applied).
b2 is folded into the h bias (b3t = W3^T b2/sqrt(A) + b3).
Scheduling notes (engine streams are in-order):
 - all input DMAs ride the sync queue (issuing from the Act sequencer
   blocks it behind HWDGE queue-depth; gpsimd DMAs thrash the GPSIMD
   ucode library),
 - the loop is software-pipelined so no mid-path wait ever head-of-line
   blocks the next tile's s0 matmuls or U multiplies,
 - gpsimd is unused: its first op after >~5us idle costs 3-9us (Q7
   cold start / library reload).

SiLU is a single Activation op on HW (use_silu=True); CoreSim has no Silu
LUT so sim validation uses Identity+Sigmoid+mul (use_silu=False).
"""

import numpy as np
import ml_dtypes
from contextlib import ExitStack

import concourse.bass as bass
import concourse.mybir as mybir
import concourse.tile as tile
from concourse import bacc
from concourse.bass_utils import run_bass_kernel_spmd

N_CORES = 8
N_FULL = 16384
NSH = N_FULL // N_CORES          # 2048 nodes per core
A = 16                           # attr dim
M0 = 128                         # MUL0 (scalar channels)
FREE = 512                       # node tile (free dim) per inner step
SCALE = 1.0 / np.sqrt(M0 * A)    # path normalization of both fctp einsums
BF16 = ml_dtypes.bfloat16

AF = mybir.ActivationFunctionType
F32 = mybir.dt.float32
DBF16 = mybir.dt.bfloat16


def build_nc(nsh: int = NSH, free: int = FREE, num_devices: int = N_CORES,
             use_silu: bool = True):
    nc = bacc.Bacc(
        "TRN2",
        target_bir_lowering=False,
        debug=False,
        enable_asserts=False,
        num_devices=num_devices,
    )

    x0r = nc.dram_tensor("x0r", [M0, 4, nsh], DBF16, kind="ExternalInput").ap()
    ar4 = nc.dram_tensor("ar4", [M0, 4, nsh], DBF16, kind="ExternalInput").ap()
    ar16 = nc.dram_tensor("ar16", [M0, 2, nsh], DBF16, kind="ExternalInput").ap()
    wt = nc.dram_tensor("wt", [M0, 16 * M0], DBF16, kind="ExternalInput").ap()
    mt = nc.dram_tensor("mt", [M0, 2 * M0], DBF16, kind="ExternalInput").ap()
    sel = nc.dram_tensor("sel", [M0, A], DBF16, kind="ExternalInput").ap()
    w4t = nc.dram_tensor("w4t", [A, 1], DBF16, kind="ExternalInput").ap()
    b1 = nc.dram_tensor("b1", [M0, 1], F32, kind="ExternalInput").ap()
    b3t = nc.dram_tensor("b3t", [A, 1], F32, kind="ExternalInput").ap()
    b4 = nc.dram_tensor("b4", [1, 1], F32, kind="ExternalInput").ap()
    outt = nc.dram_tensor("outt", [1, nsh], F32, kind="ExternalOutput").ap()

    # Tile plan: uniform chunks of `free`.  (Splitting the last chunk in
    # half to shorten the final tail chain was tried and measured WORSE —
    # the extra tile's instruction overhead outweighs the shorter chain.)
    tiles = [(i * free, free) for i in range(nsh // free)]
    n_tiles = len(tiles)

    with tile.TileContext(nc) as tc, ExitStack() as ctx:
        consts = ctx.enter_context(tc.tile_pool(name="consts", bufs=1))

        # -- engine warmups (run during the input-DMA window) ------------
        # gpsimd's first op pays a multi-us Q7 cold start; Act pays an act
        # table load.  Issue tiny dep-free ops so both happen immediately.
        warm_a = consts.tile([M0, 16], DBF16)
        warm_b = consts.tile([M0, 16], DBF16)
        warm_f = consts.tile([M0, 1], F32)
        nc.vector.memset(warm_a[:], 1.0)
        nc.vector.memset(warm_f[:], 0.0)
        warm_fn = AF.Silu if use_silu else AF.Sigmoid
        nc.scalar.activation(warm_b[:], warm_a[:], warm_fn, bias=warm_f[:])

        # -- input DMAs.  ALL on the sync HWDGE queue: the Act sequencer
        # must stay free for compute (a DMA issue blocks the issuing
        # sequencer until the HWDGE queue has space — behind a 512KB
        # transfer that is ~3.5us), and a gpsimd dma_start swaps the
        # GPSIMD ucode library.  Issue order is pipeline order: tile-t
        # U inputs first, weights woven between, small consts and the
        # later-needed ar16 slices at the end.
        xin = ctx.enter_context(tc.tile_pool(name="xin", bufs=4))
        ain = ctx.enter_context(tc.tile_pool(name="ain", bufs=4))
        gin = ctx.enter_context(tc.tile_pool(name="gin", bufs=4))
        x0s_t, a4s_t, g16_t = [], [], []
        for t, (off, sz) in enumerate(tiles):
            sl = slice(off, off + sz)
            x0s = xin.tile([M0, 4, free], DBF16, tag="x0s")
            a4s = ain.tile([M0, 4, free], DBF16, tag="a4s")
            nc.sync.dma_start(x0s[:, :, 0:sz], x0r[:, :, sl])
            nc.sync.dma_start(a4s[:, :, 0:sz], ar4[:, :, sl])
            x0s_t.append(x0s)
            a4s_t.append(a4s)
            if t == 0:
                wt_sb = consts.tile([M0, 16 * M0], DBF16)
                nc.sync.dma_start(wt_sb[:], wt)
            if t == min(1, n_tiles - 1):
                sel_sb = consts.tile([M0, A], DBF16)
                nc.sync.dma_start(sel_sb[:], sel)
                w4_sb = consts.tile([A, 1], DBF16)
                nc.sync.dma_start(w4_sb[:], w4t)
                b1_sb = consts.tile([M0, 1], F32)
                nc.sync.dma_start(b1_sb[:], b1)
                b3_sb = consts.tile([A, 1], F32)
                nc.sync.dma_start(b3_sb[:], b3t)
                b4_sb = consts.tile([1, 1], F32)
                nc.sync.dma_start(b4_sb[:], b4)
            if t == min(2, n_tiles - 1):
                # g16(0) + mid-path weights land just before rp(0) needs them
                g16 = gin.tile([M0, 2, free], DBF16, tag="g16")
                o0, s0_ = tiles[0]
                nc.sync.dma_start(g16[:, :, 0:s0_], ar16[:, :, o0:o0 + s0_])
                g16_t.append(g16)
                mt_sb = consts.tile([M0, 2 * M0], DBF16)
                nc.sync.dma_start(mt_sb[:], mt)
        while len(g16_t) < n_tiles:
            to, ts_ = tiles[len(g16_t)]
            g16 = gin.tile([M0, 2, free], DBF16, tag="g16")
            nc.sync.dma_start(g16[:, :, 0:ts_], ar16[:, :, to:to + ts_])
            g16_t.append(g16)

        upool = ctx.enter_context(tc.tile_pool(name="u", bufs=3))
        spool = ctx.enter_context(tc.tile_pool(name="s", bufs=2))
        rpool = ctx.enter_context(tc.tile_pool(name="r", bufs=2))
        hpool = ctx.enter_context(tc.tile_pool(name="h", bufs=2))
        opool = ctx.enter_context(tc.tile_pool(name="o", bufs=2))
        ps_s0 = ctx.enter_context(tc.tile_pool(name="ps_s0", bufs=2, space="PSUM"))
        ps_q = ctx.enter_context(tc.tile_pool(name="ps_q", bufs=1, space="PSUM"))
        ps_h = ctx.enter_context(tc.tile_pool(name="ps_h", bufs=2, space="PSUM"))
        ps_o = ctx.enter_context(tc.tile_pool(name="ps_o", bufs=2, space="PSUM"))


        # -- two-stage-pipelined mid path helpers -----------------------
        # stage B(t): Q' = M~^T scal (PE), staged to SBUF bf16 (Act)
        # stage C(t): R' = Q' * ar16 (DVE); h_pre = sel^T R' (PE);
        #             hb = silu (Act); out matmul (PE); store (Act queue)
        qsb_t = [None] * n_tiles

        def mid_b(t, scal):
            sz = tiles[t][1]
            q_ps = ps_q.tile([M0, 2, free], F32, tag="q")
            for b in range(2):
                nc.tensor.matmul(
                    q_ps[:, b, 0:sz], mt_sb[:, b * M0:(b + 1) * M0],
                    scal[:, 0:sz], start=True, stop=True,
                )
            if t == n_tiles - 1:
                # last tile: its q_ps is never recycled, so R' reads PSUM
                # directly on the (by-then idle) DVE — the Act staging hop
                # comes off the critical tail chain.
                qsb_t[t] = q_ps
            else:
                # single staging op for both halves (PSUM -> SBUF bf16)
                qsb = rpool.tile([M0, 2, free], DBF16, tag="qsb")
                nc.scalar.activation(qsb[:, :, 0:sz], q_ps[:, :, 0:sz],
                                     AF.Identity)
                qsb_t[t] = qsb

        h_ps_t = [None] * n_tiles

        def mid_c1(t):
            sz = tiles[t][1]
            qsb, g16 = qsb_t[t], g16_t[t]
            rp = rpool.tile([M0, 2, free], DBF16, tag="rp")
            h_ps = ps_h.tile([A, free], F32, tag="hp")
            nc.vector.tensor_mul(rp[:, :, 0:sz], qsb[:, :, 0:sz], g16[:, :, 0:sz])
            for b in range(2):
                nc.tensor.matmul(
                    h_ps[:, 0:sz], sel_sb[:], rp[:, b, 0:sz],
                    start=(b == 0), stop=(b == 1),
                )
            h_ps_t[t] = h_ps

        def mid_c2(t):
            off, sz = tiles[t]
            h_ps = h_ps_t[t]
            hb = hpool.tile([A, free], DBF16, tag="hb")
            if use_silu:
                nc.scalar.activation(hb[:, 0:sz], h_ps[:, 0:sz], AF.Silu,
                                     bias=b3_sb[:])
            else:
                hpre = hpool.tile([A, free], DBF16, tag="hpre")
                nc.scalar.activation(hpre[:, 0:sz], h_ps[:, 0:sz], AF.Identity,
                                     bias=b3_sb[:])
                hsig = hpool.tile([A, free], DBF16, tag="hsig")
                nc.scalar.activation(hsig[:, 0:sz], h_ps[:, 0:sz], AF.Sigmoid,
                                     bias=b3_sb[:])
                nc.vector.tensor_mul(hb[:, 0:sz], hpre[:, 0:sz], hsig[:, 0:sz])
            o_ps = ps_o.tile([1, free], F32, tag="op")
            nc.tensor.matmul(o_ps[:, 0:sz], w4_sb[:], hb[:, 0:sz],
                             start=True, stop=True)
            ob = opool.tile([1, free], F32, tag="ob")
            nc.scalar.activation(ob[:, 0:sz], o_ps[:, 0:sz], AF.Identity,
                                 bias=b4_sb[:])
            nc.scalar.dma_start(outt[:, off:off + sz], ob[:, 0:sz])

        # Main loop, software-pipelined two deep: engine streams are
        #   PE:  s0(0); s0(1),B(0); s0(2),B(1),C(0); s0(3),B(2),C(1); ...
        #   DVE: U(0); U(1); U(2); rp(0); U(3); rp(1); rp(2); rp(3)
        # so no stage's wait ever head-of-line blocks another tile's work.
        scal_t = [None] * n_tiles
        for t, (off, sz) in enumerate(tiles):
            x0s, a4s = x0s_t[t], a4s_t[t]

            # U chunk (q,h)[p,n] = aT[4q+p//32, n] * x0T[32h+p%32, n];
            # one conglomerate multiply per q covers h=0..3.
            u = upool.tile([M0, 16, free], DBF16, tag="u")
            for q in range(4):
                nc.vector.tensor_mul(
                    u[:, q * 4:(q + 1) * 4, 0:sz],
                    a4s[:, q:q + 1, 0:sz].broadcast_to([M0, 4, sz]),
                    x0s[:, :, 0:sz],
                )

            # s0 accumulation over the 16 c-chunks.
            s0_ps = ps_s0.tile([M0, free], F32, tag="s0")
            for g in range(16):
                nc.tensor.matmul(
                    s0_ps[:, 0:sz], wt_sb[:, g * M0:(g + 1) * M0],
                    u[:, g, 0:sz], start=(g == 0), stop=(g == 15),
                )

            # scal = silu(s0 + b1)
            scal = spool.tile([M0, free], DBF16, tag="scal")
            if use_silu:
                nc.scalar.activation(scal[:, 0:sz], s0_ps[:, 0:sz], AF.Silu,
                                     bias=b1_sb[:])
            else:
                spre = spool.tile([M0, free], DBF16, tag="spre")
                nc.scalar.activation(spre[:, 0:sz], s0_ps[:, 0:sz], AF.Identity,
                                     bias=b1_sb[:])
                ssig = spool.tile([M0, free], DBF16, tag="ssig")
                nc.scalar.activation(ssig[:, 0:sz], s0_ps[:, 0:sz], AF.Sigmoid,
                                     bias=b1_sb[:])
                nc.vector.tensor_mul(scal[:, 0:sz], spre[:, 0:sz], ssig[:, 0:sz])
            scal_t[t] = scal

            if t >= 1:
                mid_b(t - 1, scal_t[t - 1])
        mid_b(n_tiles - 1, scal_t[n_tiles - 1])
        # all R'/h/out stages after the U stream (the DVE finishes every
        # U tile first so the last tile's s0 starts as early as possible),
        # emitted stage-major in pairs so the Act/PE ping-pong of one
        # tile's tail overlaps the neighbour's instead of serializing.
        for t0_ in range(0, n_tiles, 2):
            pair = range(t0_, min(t0_ + 2, n_tiles))
            for t in pair:
                mid_c1(t)
            for t in pair:
                mid_c2(t)

    nc.compile()
    return nc


def prep_host(inputs: dict, nsh: int = NSH, n_cores: int = N_CORES):
    """Host-side prep: build the chunk-factor tensors and per-core in_maps."""
    node_vec = np.asarray(inputs["node_vec"], dtype=np.float32)
    emb = np.asarray(inputs["node_embedding"], dtype=np.float32)
    W1 = np.asarray(inputs["W1_l0"], dtype=np.float32)
    b1 = np.asarray(inputs["b1"], dtype=np.float32)
    W2 = np.asarray(inputs["W2"], dtype=np.float32)
    b2 = np.asarray(inputs["b2"], dtype=np.float32)
    W3 = np.asarray(inputs["W3"], dtype=np.float32)
    b3 = np.asarray(inputs["b3"], dtype=np.float32)
    W4 = np.asarray(inputs["W4"], dtype=np.float32)
    b4 = np.asarray(inputs["b4"], dtype=np.float32)

    N = node_vec.shape[0]
    x0T = node_vec[:, :M0].T                       # [128, N]
    aT = emb.T                                     # [16, N]

    p = np.arange(M0)
    # x0r[p, h, n] = x0T[32h + p%32, n]; ar4[p, q, n] = aT[4q + p//32, n];
    # ar16[p, b, n] = aT[8b + p//16, n]
    x0r_rows = 32 * np.arange(4)[None, :] + (p % 32)[:, None]     # [128, 4]
    ar4_rows = 4 * np.arange(4)[None, :] + (p // 32)[:, None]
    ar16_rows = 8 * np.arange(2)[None, :] + (p // 16)[:, None]
    x0r_full = x0T[x0r_rows]                       # [128, 4, N] f32
    ar4_full = aT[ar4_rows]                        # [128, 4, N]
    ar16_full = aT[ar16_rows]                      # [128, 2, N]

    # wt[p, (q*4+h)*128 + k] = W1[32h + p%32, 4q + p//32, k] * SCALE
    wt = np.empty((M0, 16, M0), dtype=np.float32)
    for q in range(4):
        for h in range(4):
            wt[:, q * 4 + h, :] = W1[32 * h + (p % 32), 4 * q + (p // 32), :M0]
    wt = (wt * SCALE).reshape(M0, 16 * M0)

    # mt[i, b*128 + a_l*16 + j] = sum_k W2[i, 8b+a_l, k]*SCALE * W3[k, j]/sqrt(A)
    m_all = np.einsum("iak,kj->iaj", W2, W3) * (SCALE / np.sqrt(A))  # [128,16,16]
    mt = m_all.reshape(M0, 2, 8 * A).reshape(M0, 2 * M0)

    # sel[p2, j] = 1 if p2 % 16 == j (sums over a_l within each half)
    sel = (p[:, None] % A == np.arange(A)[None, :]).astype(np.float32)

    w4t = (W4 / np.sqrt(A)).reshape(A, 1)
    b3t = (W3.T @ b2 / np.sqrt(A) + b3).reshape(A, 1)

    shared = {
        "wt": wt.astype(BF16),
        "mt": mt.astype(BF16),
        "sel": sel.astype(BF16),
        "w4t": w4t.astype(BF16),
        "b1": np.ascontiguousarray(b1[:M0].reshape(M0, 1)),
        "b3t": np.ascontiguousarray(b3t.astype(np.float32)),
        "b4": np.ascontiguousarray(b4.reshape(1, 1)),
    }
    in_maps = []
    for c in range(n_cores):
        csl = slice(c * nsh, (c + 1) * nsh)
        in_maps.append({
            "x0r": np.ascontiguousarray(x0r_full[:, :, csl].astype(BF16)),
            "ar4": np.ascontiguousarray(ar4_full[:, :, csl].astype(BF16)),
            "ar16": np.ascontiguousarray(ar16_full[:, :, csl].astype(BF16)),
            **shared,
        })
    return in_maps


_NC_CACHE = {}


def _get_nc():
    if "nc" not in _NC_CACHE:
        _NC_CACHE["nc"] = build_nc()
    return _NC_CACHE["nc"]


def kernel_with_results(trace: bool = False, **inputs):
    nc = _get_nc()
    in_maps = prep_host(inputs)
    res = run_bass_kernel_spmd(
        nc, in_maps, core_ids=list(range(N_CORES)), trace=trace,
    )
    out = np.empty((N_FULL, 1), dtype=np.float32)
    for c in range(N_CORES):
        out[c * NSH:(c + 1) * NSH, 0] = res.results[c]["outt"][0]
    return out, res


def kernel(**inputs) -> np.ndarray:
    out, _ = kernel_with_results(trace=False, **inputs)
    return out


# revision 52
# speedup vs baseline: 1.0139x; 1.0139x over previous
"""Trainium2 Bass kernel for nn_InvariantHeadviaTP.

Reference computation (after dead-code elimination -- y1/y2/gates are never
used by the output):
    x0   = node_vec[:, :128]                  # [N, 128]
    a    = node_embedding                     # [N, 16]
    s0   = einsum('ni,na,iak->nk', x0, a, W1_l0[:, :, :128]) / sqrt(2048) + b1[:128]
    scal = silu(s0)                           # [N, 128]
    mid  = einsum('ni,na,iak->nk', scal, a, W2) / sqrt(2048) + b2   # [N, 16]
    h    = silu(mid @ W3 / 4 + b3)            # [N, 16]
    out  = h @ W4 / 4 + b4                    # [N, 1]

Strategy: data-parallel over 8 cores (2048 nodes each), transposed layout
(features on partitions, nodes on the free dim), FREE=512 node tiles.

The bilinear contraction over c=(a,i) [16*128=2048] is chunked as 16 blocks
of (4 a's x 32 i's).  Each chunk's elementwise factor tile is then either
  ar4_q[p,n]  = aT[4q + p//32, n]   (4 distinct tiles, a replicated 32x)
  x0r_h[p,n]  = x0T[32h + p%32, n]  (4 distinct tiles, x0 rows tiled 4x)
so only 8 broadcast-inflated [128,nsh] tensors ship from HBM (4 MB/core)
instead of the 16 a-only ones (8 MB/core).  U chunks are built with 4
conglomerate DVE multiplies per tile (stride-0 / strided 3D APs), then 16
PSUM-accumulated matmuls at full 128x128 PE utilization give s0.

The mid path is reassociated: M = (W2*SCALE) @ (W3/sqrt(A)) folded on host,
Q'[(a,j),n] = M~^T scal (2 matmuls), R' = Q' * ar16 (DVE), then a one-hot
selector matmul sums over a straight into h_pre [16,n] (W3 already applied).
b2 is folded into the h bias (b3t = W3^T b2/sqrt(A) + b3).
Scheduling notes (engine streams are in-order):
 - all input DMAs ride the sync queue (issuing from the Act sequencer
   blocks it behind HWDGE queue-depth; gpsimd DMAs thrash the GPSIMD
   ucode library),
 - the loop is software-pipelined so no mid-path wait ever head-of-line
   blocks the next tile's s0 matmuls or U multiplies,
 - gpsimd is unused: its first op after >~5us idle costs 3-9us (Q7
   cold start / library reload).

SiLU is a single Activation op on HW (use_silu=True); CoreSim has no Silu
LUT so sim validation uses Identity+Sigmoid+mul (use_silu=False).
"""

import numpy as np
import ml_dtypes
from contextlib import ExitStack

import concourse.bass as bass
import concourse.mybir as mybir
import concourse.tile as tile
from concourse import bacc
from concourse.bass_utils import run_bass_kernel_spmd

N_CORES = 8
N_FULL = 16384
NSH = N_FULL // N_CORES          # 2048 nodes per core
A = 16                           # attr dim
M0 = 128                         # MUL0 (scalar channels)
FREE = 512                       # node tile (free dim) per inner step
SCALE = 1.0 / np.sqrt(M0 * A)    # path normalization of both fctp einsums
BF16 = ml_dtypes.bfloat16

AF = mybir.ActivationFunctionType
F32 = mybir.dt.float32
DBF16 = mybir.dt.bfloat16


def build_nc(nsh: int = NSH, free: int = FREE, num_devices: int = N_CORES,
             use_silu: bool = True):
    nc = bacc.Bacc(
        "TRN2",
        target_bir_lowering=False,
        debug=False,
        enable_asserts=False,
        num_devices=num_devices,
    )

    x0r = nc.dram_tensor("x0r", [M0, 4, nsh], DBF16, kind="ExternalInput").ap()
    ar4 = nc.dram_tensor("ar4", [M0, 4, nsh], DBF16, kind="ExternalInput").ap()
    ar16 = nc.dram_tensor("ar16", [M0, 2, nsh], DBF16, kind="ExternalInput").ap()
    wt = nc.dram_tensor("wt", [M0, 16 * M0], DBF16, kind="ExternalInput").ap()
    mt = nc.dram_tensor("mt", [M0, 2 * M0], DBF16, kind="ExternalInput").ap()
    sel = nc.dram_tensor("sel", [M0, A], DBF16, kind="ExternalInput").ap()
    w4t = nc.dram_tensor("w4t", [A, 1], DBF16, kind="ExternalInput").ap()
    b1 = nc.dram_tensor("b1", [M0, 1], F32, kind="ExternalInput").ap()
    b3t = nc.dram_tensor("b3t", [A, 1], F32, kind="ExternalInput").ap()
    b4 = nc.dram_tensor("b4", [1, 1], F32, kind="ExternalInput").ap()
    outt = nc.dram_tensor("outt", [1, nsh], F32, kind="ExternalOutput").ap()

    # Tile plan: uniform chunks of `free`.  (Splitting the last chunk in
    # half to shorten the final tail chain was tried and measured WORSE —
    # the extra tile's instruction overhead outweighs the shorter chain.)
    tiles = [(i * free, free) for i in range(nsh // free)]
    n_tiles = len(tiles)

    with tile.TileContext(nc) as tc, ExitStack() as ctx:
        consts = ctx.enter_context(tc.tile_pool(name="consts", bufs=1))

        # -- engine warmups (run during the input-DMA window) ------------
        # gpsimd's first op pays a multi-us Q7 cold start; Act pays an act
        # table load.  Issue tiny dep-free ops so both happen immediately.
        warm_a = consts.tile([M0, 16], DBF16)
        warm_b = consts.tile([M0, 16], DBF16)
        warm_f = consts.tile([M0, 1], F32)
        nc.vector.memset(warm_a[:], 1.0)
        nc.vector.memset(warm_f[:], 0.0)
        warm_fn = AF.Silu if use_silu else AF.Sigmoid
        nc.scalar.activation(warm_b[:], warm_a[:], warm_fn, bias=warm_f[:])

        # -- input DMAs.  ALL on the sync HWDGE queue: the Act sequencer
        # must stay free for compute (a DMA issue blocks the issuing
        # sequencer until the HWDGE queue has space — behind a 512KB
        # transfer that is ~3.5us), and a gpsimd dma_start swaps the
        # GPSIMD ucode library.  Issue order is pipeline order: tile-t
        # U inputs first, weights woven between, small consts and the
        # later-needed ar16 slices at the end.
        xin = ctx.enter_context(tc.tile_pool(name="xin", bufs=4))
        ain = ctx.enter_context(tc.tile_pool(name="ain", bufs=4))
        gin = ctx.enter_context(tc.tile_pool(name="gin", bufs=4))
        x0s_t, a4s_t, g16_t = [], [], []
        for t, (off, sz) in enumerate(tiles):
            sl = slice(off, off + sz)
            x0s = xin.tile([M0, 4, free], DBF16, tag="x0s")
            a4s = ain.tile([M0, 4, free], DBF16, tag="a4s")
            nc.sync.dma_start(x0s[:, :, 0:sz], x0r[:, :, sl])
            nc.sync.dma_start(a4s[:, :, 0:sz], ar4[:, :, sl])
            x0s_t.append(x0s)
            a4s_t.append(a4s)
            if t == 0:
                wt_sb = consts.tile([M0, 16 * M0], DBF16)
                nc.sync.dma_start(wt_sb[:], wt)
            if t == min(1, n_tiles - 1):
                sel_sb = consts.tile([M0, A], DBF16)
                nc.sync.dma_start(sel_sb[:], sel)
                w4_sb = consts.tile([A, 1], DBF16)
                nc.sync.dma_start(w4_sb[:], w4t)
                b1_sb = consts.tile([M0, 1], F32)
                nc.sync.dma_start(b1_sb[:], b1)
                b3_sb = consts.tile([A, 1], F32)
                nc.sync.dma_start(b3_sb[:], b3t)
                b4_sb = consts.tile([1, 1], F32)
                nc.sync.dma_start(b4_sb[:], b4)
            if t == min(2, n_tiles - 1):
                # g16(0) + mid-path weights land just before rp(0) needs them
                g16 = gin.tile([M0, 2, free], DBF16, tag="g16")
                o0, s0_ = tiles[0]
                nc.sync.dma_start(g16[:, :, 0:s0_], ar16[:, :, o0:o0 + s0_])
                g16_t.append(g16)
                mt_sb = consts.tile([M0, 2 * M0], DBF16)
                nc.sync.dma_start(mt_sb[:], mt)
        while len(g16_t) < n_tiles:
            to, ts_ = tiles[len(g16_t)]
            g16 = gin.tile([M0, 2, free], DBF16, tag="g16")
            nc.sync.dma_start(g16[:, :, 0:ts_], ar16[:, :, to:to + ts_])
            g16_t.append(g16)

        upool = ctx.enter_context(tc.tile_pool(name="u", bufs=3))
        spool = ctx.enter_context(tc.tile_pool(name="s", bufs=2))
        rpool = ctx.enter_context(tc.tile_pool(name="r", bufs=2))
        hpool = ctx.enter_context(tc.tile_pool(name="h", bufs=2))
        opool = ctx.enter_context(tc.tile_pool(name="o", bufs=2))
        ps_s0 = ctx.enter_context(tc.tile_pool(name="ps_s0", bufs=2, space="PSUM"))
        ps_q = ctx.enter_context(tc.tile_pool(name="ps_q", bufs=1, space="PSUM"))
        ps_h = ctx.enter_context(tc.tile_pool(name="ps_h", bufs=2, space="PSUM"))
        ps_o = ctx.enter_context(tc.tile_pool(name="ps_o", bufs=2, space="PSUM"))


        # -- two-stage-pipelined mid path helpers -----------------------
        # stage B(t): Q' = M~^T scal (PE), staged to SBUF bf16 (Act)
        # stage C(t): R' = Q' * ar16 (DVE); h_pre = sel^T R' (PE);
        #             hb = silu (Act); out matmul (PE); store (Act queue)
        qsb_t = [None] * n_tiles

        def mid_b(t, scal):
            sz = tiles[t][1]
            q_ps = ps_q.tile([M0, 2, free], F32, tag="q")
            for b in range(2):
                nc.tensor.matmul(
                    q_ps[:, b, 0:sz], mt_sb[:, b * M0:(b + 1) * M0],
                    scal[:, 0:sz], start=True, stop=True,
                )
            # single staging op for both halves (PSUM -> SBUF bf16)
            qsb = rpool.tile([M0, 2, free], DBF16, tag="qsb")
            nc.scalar.activation(qsb[:, :, 0:sz], q_ps[:, :, 0:sz], AF.Identity)
            qsb_t[t] = qsb

        h_ps_t = [None] * n_tiles

        def mid_c1(t):
            sz = tiles[t][1]
            qsb, g16 = qsb_t[t], g16_t[t]
            rp = rpool.tile([M0, 2, free], DBF16, tag="rp")
            h_ps = ps_h.tile([A, free], F32, tag="hp")
            nc.vector.tensor_mul(rp[:, :, 0:sz], qsb[:, :, 0:sz], g16[:, :, 0:sz])
            for b in range(2):
                nc.tensor.matmul(
                    h_ps[:, 0:sz], sel_sb[:], rp[:, b, 0:sz],
                    start=(b == 0), stop=(b == 1),
                )
            h_ps_t[t] = h_ps

        def mid_c2(t):
            off, sz = tiles[t]
            h_ps = h_ps_t[t]
            hb = hpool.tile([A, free], DBF16, tag="hb")
            if use_silu:
                nc.scalar.activation(hb[:, 0:sz], h_ps[:, 0:sz], AF.Silu,
                                     bias=b3_sb[:])
            else:
                hpre = hpool.tile([A, free], DBF16, tag="hpre")
                nc.scalar.activation(hpre[:, 0:sz], h_ps[:, 0:sz], AF.Identity,
                                     bias=b3_sb[:])
                hsig = hpool.tile([A, free], DBF16, tag="hsig")
                nc.scalar.activation(hsig[:, 0:sz], h_ps[:, 0:sz], AF.Sigmoid,
                                     bias=b3_sb[:])
                nc.vector.tensor_mul(hb[:, 0:sz], hpre[:, 0:sz], hsig[:, 0:sz])
            o_ps = ps_o.tile([1, free], F32, tag="op")
            nc.tensor.matmul(o_ps[:, 0:sz], w4_sb[:], hb[:, 0:sz],
                             start=True, stop=True)
            ob = opool.tile([1, free], F32, tag="ob")
            nc.scalar.activation(ob[:, 0:sz], o_ps[:, 0:sz], AF.Identity,
                                 bias=b4_sb[:])
            nc.scalar.dma_start(outt[:, off:off + sz], ob[:, 0:sz])

        # Main loop, software-pipelined two deep: engine streams are
        #   PE:  s0(0); s0(1),B(0); s0(2),B(1),C(0); s0(3),B(2),C(1); ...
        #   DVE: U(0); U(1); U(2); rp(0); U(3); rp(1); rp(2); rp(3)
        # so no stage's wait ever head-of-line blocks another tile's work.
        scal_t = [None] * n_tiles
        for t, (off, sz) in enumerate(tiles):
            x0s, a4s = x0s_t[t], a4s_t[t]

            # U chunk (q,h)[p,n] = aT[4q+p//32, n] * x0T[32h+p%32, n];
            # one conglomerate multiply per q covers h=0..3.
            u = upool.tile([M0, 16, free], DBF16, tag="u")
            for q in range(4):
                nc.vector.tensor_mul(
                    u[:, q * 4:(q + 1) * 4, 0:sz],
                    a4s[:, q:q + 1, 0:sz].broadcast_to([M0, 4, sz]),
                    x0s[:, :, 0:sz],
                )

            # s0 accumulation over the 16 c-chunks.
            s0_ps = ps_s0.tile([M0, free], F32, tag="s0")
            for g in range(16):
                nc.tensor.matmul(
                    s0_ps[:, 0:sz], wt_sb[:, g * M0:(g + 1) * M0],
                    u[:, g, 0:sz], start=(g == 0), stop=(g == 15),
                )

            # scal = silu(s0 + b1)
            scal = spool.tile([M0, free], DBF16, tag="scal")
            if use_silu:
                nc.scalar.activation(scal[:, 0:sz], s0_ps[:, 0:sz], AF.Silu,
                                     bias=b1_sb[:])
            else:
                spre = spool.tile([M0, free], DBF16, tag="spre")
                nc.scalar.activation(spre[:, 0:sz], s0_ps[:, 0:sz], AF.Identity,
                                     bias=b1_sb[:])
                ssig = spool.tile([M0, free], DBF16, tag="ssig")
                nc.scalar.activation(ssig[:, 0:sz], s0_ps[:, 0:sz], AF.Sigmoid,
                                     bias=b1_sb[:])
                nc.vector.tensor_mul(scal[:, 0:sz], spre[:, 0:sz], ssig[:, 0:sz])
            scal_t[t] = scal

            if t >= 1:
                mid_b(t - 1, scal_t[t - 1])
        mid_b(n_tiles - 1, scal_t[n_tiles - 1])
        # all R'/h/out stages after the U stream (the DVE finishes every
        # U tile first so the last tile's s0 starts as early as possible),
        # emitted stage-major in pairs so the Act/PE ping-pong of one
        # tile's tail overlaps the neighbour's instead of serializing.
        for t0_ in range(0, n_tiles, 2):
            pair = range(t0_, min(t0_ + 2, n_tiles))
            for t in pair:
                mid_c1(t)
            for t in pair:
                mid_c2(t)

    nc.compile()
    return nc


def prep_host(inputs: dict, nsh: int = NSH, n_cores: int = N_CORES):
    """Host-side prep: build the chunk-factor tensors and per-core in_maps."""
    node_vec = np.asarray(inputs["node_vec"], dtype=np.float32)
    emb = np.asarray(inputs["node_embedding"], dtype=np.float32)
    W1 = np.asarray(inputs["W1_l0"], dtype=np.float32)
    b1 = np.asarray(inputs["b1"], dtype=np.float32)
    W2 = np.asarray(inputs["W2"], dtype=np.float32)
    b2 = np.asarray(inputs["b2"], dtype=np.float32)
    W3 = np.asarray(inputs["W3"], dtype=np.float32)
    b3 = np.asarray(inputs["b3"], dtype=np.float32)
    W4 = np.asarray(inputs["W4"], dtype=np.float32)
    b4 = np.asarray(inputs["b4"], dtype=np.float32)

    N = node_vec.shape[0]
    x0T = node_vec[:, :M0].T                       # [128, N]
    aT = emb.T                                     # [16, N]

    p = np.arange(M0)
    # x0r[p, h, n] = x0T[32h + p%32, n]; ar4[p, q, n] = aT[4q + p//32, n];
    # ar16[p, b, n] = aT[8b + p//16, n]
    x0r_rows = 32 * np.arange(4)[None, :] + (p % 32)[:, None]     # [128, 4]
    ar4_rows = 4 * np.arange(4)[None, :] + (p // 32)[:, None]
    ar16_rows = 8 * np.arange(2)[None, :] + (p // 16)[:, None]
    x0r_full = x0T[x0r_rows]                       # [128, 4, N] f32
    ar4_full = aT[ar4_rows]                        # [128, 4, N]
    ar16_full = aT[ar16_rows]                      # [128, 2, N]

    # wt[p, (q*4+h)*128 + k] = W1[32h + p%32, 4q + p//32, k] * SCALE
    wt = np.empty((M0, 16, M0), dtype=np.float32)
    for q in range(4):
        for h in range(4):
            wt[:, q * 4 + h, :] = W1[32 * h + (p % 32), 4 * q + (p // 32), :M0]
    wt = (wt * SCALE).reshape(M0, 16 * M0)

    # mt[i, b*128 + a_l*16 + j] = sum_k W2[i, 8b+a_l, k]*SCALE * W3[k, j]/sqrt(A)
    m_all = np.einsum("iak,kj->iaj", W2, W3) * (SCALE / np.sqrt(A))  # [128,16,16]
    mt = m_all.reshape(M0, 2, 8 * A).reshape(M0, 2 * M0)

    # sel[p2, j] = 1 if p2 % 16 == j (sums over a_l within each half)
    sel = (p[:, None] % A == np.arange(A)[None, :]).astype(np.float32)

    w4t = (W4 / np.sqrt(A)).reshape(A, 1)
    b3t = (W3.T @ b2 / np.sqrt(A) + b3).reshape(A, 1)

    shared = {
        "wt": wt.astype(BF16),
        "mt": mt.astype(BF16),
        "sel": sel.astype(BF16),
        "w4t": w4t.astype(BF16),
        "b1": np.ascontiguousarray(b1[:M0].reshape(M0, 1)),
        "b3t": np.ascontiguousarray(b3t.astype(np.float32)),
        "b4": np.ascontiguousarray(b4.reshape(1, 1)),
    }
    in_maps = []
    for c in range(n_cores):
        csl = slice(c * nsh, (c + 1) * nsh)
        in_maps.append({
            "x0r": np.ascontiguousarray(x0r_full[:, :, csl].astype(BF16)),
            "ar4": np.ascontiguousarray(ar4_full[:, :, csl].astype(BF16)),
            "ar16": np.ascontiguousarray(ar16_full[:, :, csl].astype(BF16)),
            **shared,
        })
    return in_maps


_NC_CACHE = {}


def _get_nc():
    if "nc" not in _NC_CACHE:
        _NC_CACHE["nc"] = build_nc()
    return _NC_CACHE["nc"]


def kernel_with_results(trace: bool = False, **inputs):
    nc = _get_nc()
    in_maps = prep_host(inputs)
    res = run_bass_kernel_spmd(
        nc, in_maps, core_ids=list(range(N_CORES)), trace=trace,
    )
    out = np.empty((N_FULL, 1), dtype=np.float32)
    for c in range(N_CORES):
        out[c * NSH:(c + 1) * NSH, 0] = res.results[c]["outt"][0]
    return out, res


def kernel(**inputs) -> np.ndarray:
    out, _ = kernel_with_results(trace=False, **inputs)
    return out
